# revision 6
# baseline (speedup 1.0000x reference)
"""Trainium2 Bass kernel (fp16 device compute) for nn_CustomLinear (learned-twiddle butterfly net).

Math (validated vs reference, rel err ~1e-3):
  reference pads x [2048,4096] to [2048,8192], half-swaps (XOR N/2), then 13
  radix-2 butterfly stages with learned twiddles.  After the half-swap the lo
  half is zero, so the nonzero 4096-vector goes through:
    - stages 1..7  == one 128x128 complex matrix M per 128-block
    - stage  8     == adds only; its twiddle is FOLDED into M for odd
                     blocks (M_od = diag(tw8) @ M)
    - stages 9..12 == twiddle product on PE as diagonal matmuls; butterfly
                     sub on DVE/Pool, butterfly add via DMA accumulate or DVE
    - stage 13     == out = [t, -t], t = c13 * v: folded into the
                     transpose-out matmuls (rhs = per-block diag(c13)); only
                     t is written; the host materializes [t, -t].

Device layout: features-in-block on partitions, (block, re|im, row) on the
free dim.  z per row-tile is [128, 32*256] with each block as [re(128)|im(128)].

Engine budget per core (target ~48us each): PE diag-matmuls; DVE butterfly
subs + some adds + some copies (2x mode); Act most PSUM->SBUF copies
(1024-wide psum tiles); Pool a few subs + SWDGE accum issue; DMA input/consts/
output + ~10 accumulate-adds.

Sharding: pure data parallel, batch 2048 -> 8 cores x 256 rows.
"""
import numpy as np
from contextlib import ExitStack

import concourse.bacc as bacc
import concourse.mybir as mybir
from concourse.tile import TileContext
from concourse.bass_utils import run_bass_kernel_spmd

N = 8192
B = 2048
IN_F = 4096
NCORES = 8
B_CORE = B // NCORES          # 256 rows per core
NTILES = B_CORE // 128        # 2 row-tiles of 128 rows
NBLK = 32                     # nonzero 128-blocks per row
BF = mybir.dt.float16
F32 = mybir.dt.float32
NPBF = np.float16

PE_STAGES = (9, 10, 11, 12)

# ---- cwa column layout: M tiles (lhsT) ----
_MEVR, _MEVI, _MODR, _MODI = 0, 128, 256, 384
CWA_W = 512

# ---- cwd: per-set [Dre|Dim|mDim] for stages 9..12; d13 [mDim|Dre|Dim] ----
_NSET_S = sum(1 << (s - 8) for s in PE_STAGES)   # 30
_D13 = _NSET_S * 384
CWD_W = _D13 + NBLK * 384

# ---- tuning: butterfly-op engine assignment per (phase, tile, half) ----
# sub: "D"=DVE, "P"=Pool;  add: "D"=DVE, "M"=DMA accumulate (SWDGE)
_SUB_AS = {("pa", 0, 0): "D", ("pa", 0, 1): "D",
           ("pa", 1, 0): "P", ("pa", 1, 1): "P",
           (9, 0, 0): "D", (9, 0, 1): "D", (9, 1, 0): "D", (9, 1, 1): "D",
           (10, 0, 0): "D", (10, 0, 1): "D", (10, 1, 0): "P", (10, 1, 1): "D",
           (11, 0, 0): "D", (11, 0, 1): "D", (11, 1, 0): "P", (11, 1, 1): "D",
           (12, 0, 0): "D", (12, 0, 1): "D", (12, 1, 0): "D", (12, 1, 1): "D"}
_ADD_AS = {("pa", 0, 0): "M", ("pa", 0, 1): "M",
           ("pa", 1, 0): "M", ("pa", 1, 1): "M",
           (9, 0, 0): "D", (9, 0, 1): "D", (9, 1, 0): "D", (9, 1, 1): "D",
           (10, 0, 0): "D", (10, 0, 1): "D", (10, 1, 0): "D", (10, 1, 1): "D",
           (11, 0, 0): "M", (11, 0, 1): "M", (11, 1, 0): "M", (11, 1, 1): "M",
           (12, 0, 0): "M", (12, 0, 1): "M", (12, 1, 0): "D", (12, 1, 1): "D"}
# copies: Act-heavy with periodic DVE (ratio ~5:2)
_CP_PAT = "AADAADA"

_CACHE = {}


def _stage_tw(s, w):
    step = 1 << s
    half = step >> 1
    k = np.arange(half) * (N // step)
    ang = (-2.0 * np.pi / N) * k.astype(np.float64) * w[k].astype(np.float64)
    return np.exp(1j * ang)


def _host_consts(w):
    M = np.eye(128, dtype=np.complex128)
    for s in range(1, 8):
        step = 1 << s
        half = step >> 1
        tw = _stage_tw(s, w)
        Bm = np.zeros((step, step), np.complex128)
        Bm[:half, :half] = np.eye(half)
        Bm[:half, half:] = np.diag(tw)
        Bm[half:, :half] = np.eye(half)
        Bm[half:, half:] = -np.diag(tw)
        M = np.kron(np.eye(128 // step), Bm) @ M
    tw8 = _stage_tw(8, w)
    M_od = np.diag(tw8) @ M

    cwa = np.zeros((128, CWA_W), np.float32)
    # lhsT tiles: lhsT[e, e'] = M[e', e]  (out = lhsT.T @ rhs = M @ rhs)
    cwa[:, _MEVR:_MEVR + 128] = M.real.T
    cwa[:, _MEVI:_MEVI + 128] = M.imag.T
    cwa[:, _MODR:_MODR + 128] = M_od.real.T
    cwa[:, _MODI:_MODI + 128] = M_od.imag.T

    cwd = np.zeros((128, CWD_W), np.float32)
    ii = np.arange(128)
    off = 0
    for s in PE_STAGES:
        tw = _stage_tw(s, w)
        for jr in range(1 << (s - 8)):
            dre = tw.real[jr * 128:(jr + 1) * 128]
            dim = tw.imag[jr * 128:(jr + 1) * 128]
            cwd[ii, off + ii] = dre
            cwd[ii, off + 128 + ii] = dim
            cwd[ii, off + 256 + ii] = -dim
            off += 384
    c13 = _stage_tw(13, w)
    for j in range(NBLK):
        dre = c13.real[j * 128:(j + 1) * 128]
        dim = c13.imag[j * 128:(j + 1) * 128]
        # [mDim | Dre | Dim]: d1 = cols 128..384 = [Dre|Dim],
        #                     d2 = cols   0..256 = [mDim|Dre]
        cwd[ii, off + ii] = -dim
        cwd[ii, off + 128 + ii] = dre
        cwd[ii, off + 256 + ii] = dim
        off += 384
    return cwa.astype(NPBF), cwd.astype(NPBF)


def _set_off(s, jr):
    off = 0
    for t in PE_STAGES:
        if t == s:
            return off + jr * 384
        off += (1 << (t - 8)) * 384
    raise ValueError(s)


def _build_program():
    nc = bacc.Bacc("TRN2", target_bir_lowering=False, debug=False)
    x_d = nc.dram_tensor("x", [B_CORE, IN_F], BF, kind="ExternalInput").ap()
    cwa_d = nc.dram_tensor("cwa", [128, CWA_W], BF, kind="ExternalInput").ap()
    cwd_d = nc.dram_tensor("cwd", [128, CWD_W], BF, kind="ExternalInput").ap()
    y_d = nc.dram_tensor("y", [B_CORE, 2 * IN_F], BF, kind="ExternalOutput").ap()

    AL = mybir.AluOpType
    cp_state = [0]

    with TileContext(nc) as tc, ExitStack() as ctx:
        cpool = ctx.enter_context(tc.tile_pool(name="const", bufs=1))
        xtpool = ctx.enter_context(tc.tile_pool(name="xt", bufs=1))
        zpool = ctx.enter_context(tc.tile_pool(name="z", bufs=1))
        tpool = ctx.enter_context(tc.tile_pool(name="t", bufs=2))
        opool = ctx.enter_context(tc.tile_pool(name="out", bufs=4))
        ps = ctx.enter_context(tc.tile_pool(name="ps", bufs=4, space="PSUM"))

        def copy(dst, src):
            eng = _CP_PAT[cp_state[0] % len(_CP_PAT)]
            cp_state[0] += 1
            if eng == "A":
                nc.scalar.copy(dst, src)
            else:
                nc.vector.tensor_copy(dst, src)

        def bsub(key, dst, in0, in1):
            eng = nc.vector if _SUB_AS[key] == "D" else nc.gpsimd
            eng.tensor_tensor(dst, in0, in1, op=AL.subtract)

        def badd(key, dst, src):
            # dst += src
            if _ADD_AS[key] == "M":
                nc.gpsimd.dma_start(dst, src, accum_op=AL.add)
            else:
                nc.vector.tensor_tensor(dst, dst, src, op=AL.add)

        # ---- DMAs, ordered by first use ----
        cwa = cpool.tile([128, CWA_W], BF)
        nc.sync.dma_start(cwa[:], cwa_d[:])
        mevr = cwa[:, _MEVR:_MEVR + 128]
        mevi = cwa[:, _MEVI:_MEVI + 128]
        modr = cwa[:, _MODR:_MODR + 128]
        modi = cwa[:, _MODI:_MODI + 128]

        cwd = cpool.tile([128, CWD_W], BF)
        xts = []
        for ti in range(NTILES):
            xt = xtpool.tile([128, IN_F], BF, tag=f"xt{ti}")
            xts.append(xt)

        def load_x(ti, q0, nq):
            # transpose-load quarters q0..q0+nq of tile ti (8 blocks each)
            xt = xts[ti]
            c0, c1 = q0 * 1024, (q0 + nq) * 1024
            xv3 = xt[:, c0:c1].rearrange("p (j r) -> p j r", j=8 * nq)
            nc.sync.dma_start_transpose(
                xv3, x_d[ti * 128:ti * 128 + 128, c0:c1])

        _o91 = _set_off(9, 0)
        _o11 = _set_off(11, 0)
        _o12 = _set_off(12, 0)

        # warmup matmuls: keep PE busy during input DMA so the p-state ramps
        wps = ps.tile([128, 1024], F32, tag="ps")
        for _ in range(10):
            nc.tensor.matmul(wps[:, 0:512], mevr, cwa[:, 0:512],
                             start=True, stop=True)
        wsc = cpool.tile([128, 8], BF)
        nc.scalar.copy(wsc[:], wps[:, 0:8])

        load_x(0, 0, 1)
        load_x(0, 1, 1)
        nc.sync.dma_start(cwd[:, _o91:_o11], cwd_d[:, _o91:_o11])   # s9+s10
        load_x(0, 2, 1)
        load_x(0, 3, 1)
        load_x(1, 0, 2)
        nc.sync.dma_start(cwd[:, _o11:_o12], cwd_d[:, _o11:_o12])   # s11
        load_x(1, 2, 2)
        nc.sync.dma_start(cwd[:, _o12:_D13], cwd_d[:, _o12:_D13])   # s12
        nc.sync.dma_start(cwd[:, _D13:_D13 + 16 * 384],
                          cwd_d[:, _D13:_D13 + 16 * 384])           # d13 a
        nc.sync.dma_start(cwd[:, _D13 + 16 * 384:],
                          cwd_d[:, _D13 + 16 * 384:])               # d13 b

        def dset(s, jr):
            o = _set_off(s, jr)
            return (cwd[:, o:o + 128],          # Dre
                    cwd[:, o + 128:o + 256],    # Dim
                    cwd[:, o + 256:o + 384])    # -Dim

        def d13(j):
            o = _D13 + j * 384
            return cwd[:, o + 128:o + 384], cwd[:, o:o + 256]  # d1, d2

        zs = []
        for ti in range(NTILES):
            zt = zpool.tile([128, NBLK * 256], BF, tag=f"z{ti}")
            zs.append(zt)

        def make_tile(ti):
            st = {}
            r0 = ti * 128
            z = zs[ti]
            z3 = z[:].rearrange("p (j c) -> p j c", j=NBLK)       # [p,32,256]

            def pa(h):
                """stages 1..8 for half h (8 block-pairs)."""
                xt = xts[ti]
                xv = xt[:].rearrange("p (g two r) -> p g two r", g=16, two=2)
                if h == 0:
                    t8 = tpool.tile([128, 16 * 256], BF, tag=f"t{ti}")
                    st["t8"] = t8
                t8 = st["t8"]
                t8v = t8[:].rearrange("p (g c) -> p g c", g=16)
                zvp = z[:].rearrange("p (g two c) -> p g two c", g=16, two=2)
                for c in range(2):                  # chunks of 4 pairs
                    g0 = h * 8 + c * 4
                    rev = xv[:, g0:g0 + 4, 0, :]
                    rod = xv[:, g0:g0 + 4, 1, :]
                    p_ev = ps.tile([128, 1024], F32, tag="ps")
                    p_od = ps.tile([128, 1024], F32, tag="ps")
                    nc.tensor.matmul(p_ev[:, 0:512], mevr, rev,
                                     start=True, stop=True)
                    nc.tensor.matmul(p_ev[:, 512:1024], mevi, rev,
                                     start=True, stop=True)
                    nc.tensor.matmul(p_od[:, 0:512], modr, rod,
                                     start=True, stop=True)
                    nc.tensor.matmul(p_od[:, 512:1024], modi, rod,
                                     start=True, stop=True)
                    # psum layout [part(2), g(4), r(128)] -> interleaved dst
                    pev = p_ev[:].rearrange("p (t g r) -> p t g r",
                                            t=2, g=4, r=128)
                    pod = p_od[:].rearrange("p (t g r) -> p t g r",
                                            t=2, g=4, r=128)
                    zev = zvp[:, g0:g0 + 4, 0, :].rearrange(
                        "p g (t r) -> p t g r", t=2, r=128)
                    t8d = t8v[:, g0:g0 + 4, :].rearrange(
                        "p g (t r) -> p t g r", t=2, r=128)
                    copy(zev, pev)
                    copy(t8d, pod)
                # stage-8 butterfly: z[2g+1] = z[2g] - t8 ; z[2g] += t8
                g0 = h * 8
                zev = zvp[:, g0:g0 + 8, 0, :]
                zod = zvp[:, g0:g0 + 8, 1, :]
                t8s = t8v[:, g0:g0 + 8, :]
                bsub(("pa", ti, h), zod, zev, t8s)
                badd(("pa", ti, h), zev, t8s)

            def stage(s, h):
                G = 1 << (s - 7)
                hb = G // 2
                ng = NBLK // G
                gh = max(1, ng // 2)
                z5 = z[:].rearrange("p (g G c) -> p g G c", g=ng, G=G)
                if h == 0:
                    t = tpool.tile([128, 16 * 256], BF, tag=f"t{ti}")
                    st["t"] = t
                t = st["t"]
                # t layout: [p, hb, ng, 256]
                tv = t[:].rearrange("p (j g c) -> p j g c", j=hb, g=ng, c=256)
                if s < 12:
                    g0, g1 = h * gh, (h + 1) * gh
                    jrs = list(range(hb))
                else:
                    g0, g1 = 0, 1
                    jrs = list(range(h * 8, h * 8 + 8))
                w_ = (g1 - g0) * 128
                p_re = ps.tile([128, 1024], F32, tag="ps")
                p_im = ps.tile([128, 1024], F32, tag="ps")
                for k, jr in enumerate(jrs):
                    dre, dim, mdim = dset(s, jr)
                    hr = z5[:, g0:g1, hb + jr, 0:128]
                    hi = z5[:, g0:g1, hb + jr, 128:256]
                    tr = p_re[:, k * w_:(k + 1) * w_]
                    tim = p_im[:, k * w_:(k + 1) * w_]
                    nc.tensor.matmul(tr, dre, hr, start=True, stop=False)
                    nc.tensor.matmul(tr, mdim, hi, start=False, stop=True)
                    nc.tensor.matmul(tim, dim, hr, start=True, stop=False)
                    nc.tensor.matmul(tim, dre, hi, start=False, stop=True)
                # copies: psum [jr, g, r] -> tv[:, jr, g, part*128:+128]
                prv = p_re[:].rearrange("p (j g r) -> p j g r",
                                        j=len(jrs), g=g1 - g0, r=128)
                piv = p_im[:].rearrange("p (j g r) -> p j g r",
                                        j=len(jrs), g=g1 - g0, r=128)
                if s < 12:
                    copy(tv[:, :, g0:g1, 0:128], prv)
                    copy(tv[:, :, g0:g1, 128:256], piv)
                    lo = z5[:, g0:g1, 0:hb, :]
                    hi_ = z5[:, g0:g1, hb:G, :]
                    tt = tv[:].rearrange("p j g c -> p g j c")[:, g0:g1, :, :]
                else:
                    j0 = h * 8
                    copy(tv[:, j0:j0 + 8, 0, 0:128], prv)
                    copy(tv[:, j0:j0 + 8, 0, 128:256], piv)
                    lo = z3[:, j0:j0 + 8, :]
                    hi_ = z3[:, 16 + j0:16 + j0 + 8, :]
                    tt = tv[:, j0:j0 + 8, 0, :]
                bsub((s, ti, h), hi_, lo, tt)
                badd((s, ti, h), lo, tt)

            def out(q):
                """transpose-out + c13 for quarter q (8 blocks)."""
                jb = q * 8
                ob = opool.tile([128, 2048], BF, tag="ob")
                for pt in range(2):
                    pm = ps.tile([128, 1024], F32, tag="ps")
                    for k in range(4):
                        j = jb + pt * 4 + k
                        da, db = d13(j)
                        tgt = pm[:, k * 256:k * 256 + 256]
                        nc.tensor.matmul(tgt, z3[:, j, 0:128], da,
                                         start=True, stop=False)
                        nc.tensor.matmul(tgt, z3[:, j, 128:256], db,
                                         start=False, stop=True)
                    copy(ob[:, pt * 1024:pt * 1024 + 1024], pm[:])
                nc.sync.dma_start(
                    y_d[r0:r0 + 128, q * 2048:(q + 1) * 2048], ob[:])

            return {"pa": pa, "stage": stage, "out": out}

        t0 = make_tile(0)
        t1 = make_tile(1)
        # software-pipelined emission at half-phase granularity
        t0["pa"](0)
        t0["pa"](1)
        t0["stage"](9, 0)
        t0["stage"](9, 1)
        t1["pa"](0)
        t0["stage"](10, 0)
        t1["pa"](1)
        t0["stage"](10, 1)
        t1["stage"](9, 0)
        t0["stage"](11, 0)
        t1["stage"](9, 1)
        t0["stage"](11, 1)
        t1["stage"](10, 0)
        t0["stage"](12, 0)
        t1["stage"](10, 1)
        t0["stage"](12, 1)
        t1["stage"](11, 0)
        t0["out"](0)
        t0["out"](1)
        t1["stage"](11, 1)
        t0["out"](2)
        t1["stage"](12, 0)
        t0["out"](3)
        t1["stage"](12, 1)
        t1["out"](0)
        t1["out"](1)
        t1["out"](2)
        t1["out"](3)

    nc.compile()
    return nc


def kernel(x: np.ndarray, weights: np.ndarray) -> np.ndarray:
    x = np.asarray(x, dtype=np.float32)
    w = np.asarray(weights, dtype=np.float32)
    xb = np.ascontiguousarray(x.astype(NPBF))
    if "nc" not in _CACHE:
        _CACHE["nc"] = _build_program()
    nc = _CACHE["nc"]
    cwa, cwd = _host_consts(w)
    in_maps = [
        {"x": xb[ci * B_CORE:(ci + 1) * B_CORE], "cwa": cwa, "cwd": cwd}
        for ci in range(NCORES)
    ]
    res = run_bass_kernel_spmd(nc, in_maps, list(range(NCORES)))
    _CACHE["last_results"] = res
    t = np.concatenate([res.results[ci]["y"] for ci in range(NCORES)], axis=0)
    # y row layout: 32 x [re_block(128) | im_block(128)], block-major
    t = t.astype(np.float32).reshape(B, NBLK, 2, 128)
    t = (t[:, :, 0, :] + 1j * t[:, :, 1, :]).astype(np.complex64)
    t = t.reshape(B, IN_F)
    return np.concatenate([t, -t], axis=1)           # [2048, 8192]


# revision 8
# speedup vs baseline: 1.2931x; 1.2931x over previous
"""Trainium2 Bass kernel (fp16 device compute) for nn_CustomLinear (learned-twiddle butterfly net).

Math (validated vs reference, rel err ~1e-3):
  reference pads x [2048,4096] to [2048,8192], half-swaps (XOR N/2), then 13
  radix-2 butterfly stages with learned twiddles.  After the half-swap the lo
  half is zero, so the nonzero 4096-vector goes through:
    - stages 1..7  == one 128x128 complex matrix M per 128-block
    - stage  8     == adds only; its twiddle is FOLDED into M for odd
                     blocks (M_od = diag(tw8) @ M)
    - stages 9..12 == twiddle product on PE as diagonal matmuls; butterfly
                     sub on DVE/Pool, butterfly add via DMA accumulate or DVE
    - stage 13     == out = [t, -t], t = c13 * v: folded into the
                     transpose-out matmuls (rhs = per-block diag(c13)); only
                     t is written; the host materializes [t, -t].

Device layout: features-in-block on partitions, (block, re|im, row) on the
free dim.  z per row-tile is [128, 32*256] with each block as [re(128)|im(128)].

Engine budget per core (target ~48us each): PE diag-matmuls; DVE butterfly
subs + some adds + some copies (2x mode); Act most PSUM->SBUF copies
(1024-wide psum tiles); Pool a few subs + SWDGE accum issue; DMA input/consts/
output + ~10 accumulate-adds.

Sharding: pure data parallel, batch 2048 -> 8 cores x 256 rows.
"""
import numpy as np
from contextlib import ExitStack

import concourse.bacc as bacc
import concourse.mybir as mybir
from concourse.tile import TileContext
from concourse.bass_utils import run_bass_kernel_spmd

N = 8192
B = 2048
IN_F = 4096
NCORES = 8
B_CORE = B // NCORES          # 256 rows per core
NTILES = B_CORE // 128        # 2 row-tiles of 128 rows
NBLK = 32                     # nonzero 128-blocks per row
BF = mybir.dt.float16
F32 = mybir.dt.float32
NPBF = np.float16

PE_STAGES = (9, 10, 11, 12)

# ---- cwa column layout: M tiles (lhsT) ----
_MEVR, _MEVI, _MODR, _MODI = 0, 128, 256, 384
CWA_W = 512

# ---- cwd: per-set [Dre|Dim|mDim] for stages 9..12; d13 [mDim|Dre|Dim] ----
_NSET_S = sum(1 << (s - 8) for s in PE_STAGES)   # 30
_D13 = _NSET_S * 384
CWD_W = _D13 + NBLK * 384

# ---- tuning: butterfly-op engine assignment per (phase, tile, half) ----
# sub: "D"=DVE, "P"=Pool;  add: "D"=DVE, "M"=DMA accumulate (SWDGE)
_SUB_AS = {("pa", 0, 0): "D", ("pa", 0, 1): "D",
           ("pa", 1, 0): "P", ("pa", 1, 1): "P",
           (9, 0, 0): "D", (9, 0, 1): "D", (9, 1, 0): "D", (9, 1, 1): "D",
           (10, 0, 0): "D", (10, 0, 1): "D", (10, 1, 0): "P", (10, 1, 1): "D",
           (11, 0, 0): "D", (11, 0, 1): "D", (11, 1, 0): "P", (11, 1, 1): "D",
           (12, 0, 0): "D", (12, 0, 1): "D", (12, 1, 0): "D", (12, 1, 1): "D"}
_ADD_AS = {("pa", 0, 0): "M", ("pa", 0, 1): "M",
           ("pa", 1, 0): "M", ("pa", 1, 1): "M",
           (9, 0, 0): "D", (9, 0, 1): "D", (9, 1, 0): "D", (9, 1, 1): "D",
           (10, 0, 0): "D", (10, 0, 1): "D", (10, 1, 0): "D", (10, 1, 1): "D",
           (11, 0, 0): "M", (11, 0, 1): "M", (11, 1, 0): "M", (11, 1, 1): "M",
           (12, 0, 0): "M", (12, 0, 1): "M", (12, 1, 0): "D", (12, 1, 1): "D"}
# copies: Act-heavy with periodic DVE (ratio ~5:2)
_CP_PAT = "AADAADA"

_CACHE = {}


def _stage_tw(s, w):
    step = 1 << s
    half = step >> 1
    k = np.arange(half) * (N // step)
    ang = (-2.0 * np.pi / N) * k.astype(np.float64) * w[k].astype(np.float64)
    return np.exp(1j * ang)


def _host_consts(w):
    M = np.eye(128, dtype=np.complex128)
    for s in range(1, 8):
        step = 1 << s
        half = step >> 1
        tw = _stage_tw(s, w)
        Bm = np.zeros((step, step), np.complex128)
        Bm[:half, :half] = np.eye(half)
        Bm[:half, half:] = np.diag(tw)
        Bm[half:, :half] = np.eye(half)
        Bm[half:, half:] = -np.diag(tw)
        M = np.kron(np.eye(128 // step), Bm) @ M
    tw8 = _stage_tw(8, w)
    M_od = np.diag(tw8) @ M

    cwa = np.zeros((128, CWA_W), np.float32)
    # lhsT tiles: lhsT[e, e'] = M[e', e]  (out = lhsT.T @ rhs = M @ rhs)
    cwa[:, _MEVR:_MEVR + 128] = M.real.T
    cwa[:, _MEVI:_MEVI + 128] = M.imag.T
    cwa[:, _MODR:_MODR + 128] = M_od.real.T
    cwa[:, _MODI:_MODI + 128] = M_od.imag.T

    cwd = np.zeros((128, CWD_W), np.float32)
    ii = np.arange(128)
    off = 0
    for s in PE_STAGES:
        tw = _stage_tw(s, w)
        for jr in range(1 << (s - 8)):
            dre = tw.real[jr * 128:(jr + 1) * 128]
            dim = tw.imag[jr * 128:(jr + 1) * 128]
            cwd[ii, off + ii] = dre
            cwd[ii, off + 128 + ii] = dim
            cwd[ii, off + 256 + ii] = -dim
            off += 384
    c13 = _stage_tw(13, w)
    for j in range(NBLK):
        dre = c13.real[j * 128:(j + 1) * 128]
        dim = c13.imag[j * 128:(j + 1) * 128]
        # [mDim | Dre | Dim]: d1 = cols 128..384 = [Dre|Dim],
        #                     d2 = cols   0..256 = [mDim|Dre]
        cwd[ii, off + ii] = -dim
        cwd[ii, off + 128 + ii] = dre
        cwd[ii, off + 256 + ii] = dim
        off += 384
    return cwa.astype(NPBF), cwd.astype(NPBF)


def _set_off(s, jr):
    off = 0
    for t in PE_STAGES:
        if t == s:
            return off + jr * 384
        off += (1 << (t - 8)) * 384
    raise ValueError(s)


def _build_program():
    nc = bacc.Bacc("TRN2", target_bir_lowering=False, debug=False)
    x_d = nc.dram_tensor("x", [B_CORE, IN_F], BF, kind="ExternalInput").ap()
    cwa_d = nc.dram_tensor("cwa", [128, CWA_W], BF, kind="ExternalInput").ap()
    cwd_d = nc.dram_tensor("cwd", [128, CWD_W], BF, kind="ExternalInput").ap()
    y_d = nc.dram_tensor("y", [B_CORE, 2 * IN_F], BF, kind="ExternalOutput").ap()

    AL = mybir.AluOpType
    cp_state = [0]

    with TileContext(nc) as tc, ExitStack() as ctx:
        cpool = ctx.enter_context(tc.tile_pool(name="const", bufs=1))
        xtpool = ctx.enter_context(tc.tile_pool(name="xt", bufs=1))
        zpool = ctx.enter_context(tc.tile_pool(name="z", bufs=1))
        tpool = ctx.enter_context(tc.tile_pool(name="t", bufs=2))
        opool = ctx.enter_context(tc.tile_pool(name="out", bufs=4))
        ps = ctx.enter_context(tc.tile_pool(name="ps", bufs=4, space="PSUM"))

        def copy(dst, src):
            eng = _CP_PAT[cp_state[0] % len(_CP_PAT)]
            cp_state[0] += 1
            if eng == "A":
                nc.scalar.copy(dst, src)
            else:
                nc.vector.tensor_copy(dst, src)

        def bsub(key, dst, in0, in1):
            eng = nc.vector if _SUB_AS[key] == "D" else nc.gpsimd
            eng.tensor_tensor(dst, in0, in1, op=AL.subtract)

        def badd(key, dst, src):
            # dst += src
            if _ADD_AS[key] == "M":
                nc.gpsimd.dma_start(dst, src, accum_op=AL.add)
            else:
                nc.vector.tensor_tensor(dst, dst, src, op=AL.add)

        # ---- DMAs, ordered by first use ----
        cwa = cpool.tile([128, CWA_W], BF)
        nc.sync.dma_start(cwa[:], cwa_d[:])
        mevr = cwa[:, _MEVR:_MEVR + 128]
        mevi = cwa[:, _MEVI:_MEVI + 128]
        modr = cwa[:, _MODR:_MODR + 128]
        modi = cwa[:, _MODI:_MODI + 128]

        cwd = cpool.tile([128, CWD_W], BF)
        xts = []
        for ti in range(NTILES):
            xt = xtpool.tile([128, IN_F], BF, tag=f"xt{ti}")
            xts.append(xt)

        def load_x(ti, q0, nq):
            # transpose-load quarters q0..q0+nq of tile ti (8 blocks each)
            xt = xts[ti]
            c0, c1 = q0 * 1024, (q0 + nq) * 1024
            xv3 = xt[:, c0:c1].rearrange("p (j r) -> p j r", j=8 * nq)
            nc.sync.dma_start_transpose(
                xv3, x_d[ti * 128:ti * 128 + 128, c0:c1])

        _o91 = _set_off(9, 0)
        _o11 = _set_off(11, 0)
        _o12 = _set_off(12, 0)

        # warmup matmuls: keep PE busy during input DMA so the p-state ramps
        wps = ps.tile([128, 1024], F32, tag="ps")
        for _ in range(4):
            nc.tensor.matmul(wps[:, 0:512], mevr, cwa[:, 0:512],
                             start=True, stop=True)
        wsc = cpool.tile([128, 8], BF)
        nc.scalar.copy(wsc[:], wps[:, 0:8])

        load_x(0, 0, 1)
        load_x(0, 1, 1)
        nc.sync.dma_start(cwd[:, _o91:_o11], cwd_d[:, _o91:_o11])   # s9+s10
        load_x(0, 2, 1)
        load_x(0, 3, 1)
        load_x(1, 0, 2)
        nc.sync.dma_start(cwd[:, _o11:_o12], cwd_d[:, _o11:_o12])   # s11
        load_x(1, 2, 2)
        nc.sync.dma_start(cwd[:, _o12:_D13], cwd_d[:, _o12:_D13])   # s12
        nc.sync.dma_start(cwd[:, _D13:_D13 + 16 * 384],
                          cwd_d[:, _D13:_D13 + 16 * 384])           # d13 a
        nc.sync.dma_start(cwd[:, _D13 + 16 * 384:],
                          cwd_d[:, _D13 + 16 * 384:])               # d13 b

        def dset(s, jr):
            o = _set_off(s, jr)
            return (cwd[:, o:o + 128],          # Dre
                    cwd[:, o + 128:o + 256],    # Dim
                    cwd[:, o + 256:o + 384])    # -Dim

        def d13(j):
            o = _D13 + j * 384
            return cwd[:, o + 128:o + 384], cwd[:, o:o + 256]  # d1, d2

        zs = []
        for ti in range(NTILES):
            zt = zpool.tile([128, NBLK * 256], BF, tag=f"z{ti}")
            zs.append(zt)

        def make_tile(ti):
            st = {}
            r0 = ti * 128
            z = zs[ti]
            z3 = z[:].rearrange("p (j c) -> p j c", j=NBLK)       # [p,32,256]

            def pa(h):
                """stages 1..8 for half h (8 block-pairs)."""
                xt = xts[ti]
                xv = xt[:].rearrange("p (g two r) -> p g two r", g=16, two=2)
                if h == 0:
                    t8 = tpool.tile([128, 16 * 256], BF, tag=f"t{ti}")
                    st["t8"] = t8
                t8 = st["t8"]
                t8v = t8[:].rearrange("p (g c) -> p g c", g=16)
                zvp = z[:].rearrange("p (g two c) -> p g two c", g=16, two=2)
                for c in range(2):                  # chunks of 4 pairs
                    g0 = h * 8 + c * 4
                    rev = xv[:, g0:g0 + 4, 0, :]
                    rod = xv[:, g0:g0 + 4, 1, :]
                    p_ev = ps.tile([128, 1024], F32, tag="ps")
                    p_od = ps.tile([128, 1024], F32, tag="ps")
                    nc.tensor.matmul(p_ev[:, 0:512], mevr, rev,
                                     start=True, stop=True)
                    nc.tensor.matmul(p_ev[:, 512:1024], mevi, rev,
                                     start=True, stop=True)
                    nc.tensor.matmul(p_od[:, 0:512], modr, rod,
                                     start=True, stop=True)
                    nc.tensor.matmul(p_od[:, 512:1024], modi, rod,
                                     start=True, stop=True)
                    # psum layout [part(2), g(4), r(128)] -> interleaved dst
                    pev = p_ev[:].rearrange("p (t g r) -> p t g r",
                                            t=2, g=4, r=128)
                    pod = p_od[:].rearrange("p (t g r) -> p t g r",
                                            t=2, g=4, r=128)
                    zev = zvp[:, g0:g0 + 4, 0, :].rearrange(
                        "p g (t r) -> p t g r", t=2, r=128)
                    t8d = t8v[:, g0:g0 + 4, :].rearrange(
                        "p g (t r) -> p t g r", t=2, r=128)
                    copy(zev, pev)
                    copy(t8d, pod)
                # stage-8 butterfly: z[2g+1] = z[2g] - t8 ; z[2g] += t8
                g0 = h * 8
                zev = zvp[:, g0:g0 + 8, 0, :]
                zod = zvp[:, g0:g0 + 8, 1, :]
                t8s = t8v[:, g0:g0 + 8, :]
                bsub(("pa", ti, h), zod, zev, t8s)
                badd(("pa", ti, h), zev, t8s)

            def stage(s, h):
                G = 1 << (s - 7)
                hb = G // 2
                ng = NBLK // G
                gh = max(1, ng // 2)
                z5 = z[:].rearrange("p (g G c) -> p g G c", g=ng, G=G)
                if h == 0:
                    t = tpool.tile([128, 16 * 256], BF, tag=f"t{ti}")
                    st["t"] = t
                t = st["t"]
                # t layout: [p, hb, ng, 256]
                tv = t[:].rearrange("p (j g c) -> p j g c", j=hb, g=ng, c=256)
                if s < 12:
                    g0, g1 = h * gh, (h + 1) * gh
                    jrs = list(range(hb))
                else:
                    g0, g1 = 0, 1
                    jrs = list(range(h * 8, h * 8 + 8))
                w_ = (g1 - g0) * 128
                p_re = ps.tile([128, 1024], F32, tag="ps")
                p_im = ps.tile([128, 1024], F32, tag="ps")
                for k, jr in enumerate(jrs):
                    dre, dim, mdim = dset(s, jr)
                    hr = z5[:, g0:g1, hb + jr, 0:128]
                    hi = z5[:, g0:g1, hb + jr, 128:256]
                    tr = p_re[:, k * w_:(k + 1) * w_]
                    tim = p_im[:, k * w_:(k + 1) * w_]
                    nc.tensor.matmul(tr, dre, hr, start=True, stop=False)
                    nc.tensor.matmul(tr, mdim, hi, start=False, stop=True)
                    nc.tensor.matmul(tim, dim, hr, start=True, stop=False)
                    nc.tensor.matmul(tim, dre, hi, start=False, stop=True)
                # copies: psum [jr, g, r] -> tv[:, jr, g, part*128:+128]
                prv = p_re[:].rearrange("p (j g r) -> p j g r",
                                        j=len(jrs), g=g1 - g0, r=128)
                piv = p_im[:].rearrange("p (j g r) -> p j g r",
                                        j=len(jrs), g=g1 - g0, r=128)
                if s < 12:
                    copy(tv[:, :, g0:g1, 0:128], prv)
                    copy(tv[:, :, g0:g1, 128:256], piv)
                    lo = z5[:, g0:g1, 0:hb, :]
                    hi_ = z5[:, g0:g1, hb:G, :]
                    tt = tv[:].rearrange("p j g c -> p g j c")[:, g0:g1, :, :]
                else:
                    j0 = h * 8
                    copy(tv[:, j0:j0 + 8, 0, 0:128], prv)
                    copy(tv[:, j0:j0 + 8, 0, 128:256], piv)
                    lo = z3[:, j0:j0 + 8, :]
                    hi_ = z3[:, 16 + j0:16 + j0 + 8, :]
                    tt = tv[:, j0:j0 + 8, 0, :]
                bsub((s, ti, h), hi_, lo, tt)
                badd((s, ti, h), lo, tt)

            def out(q):
                """transpose-out + c13 for quarter q (8 blocks)."""
                jb = q * 8
                ob = opool.tile([128, 2048], BF, tag="ob")
                for pt in range(2):
                    pm = ps.tile([128, 1024], F32, tag="ps")
                    for k in range(4):
                        j = jb + pt * 4 + k
                        da, db = d13(j)
                        tgt = pm[:, k * 256:k * 256 + 256]
                        nc.tensor.matmul(tgt, z3[:, j, 0:128], da,
                                         start=True, stop=False)
                        nc.tensor.matmul(tgt, z3[:, j, 128:256], db,
                                         start=False, stop=True)
                    copy(ob[:, pt * 1024:pt * 1024 + 1024], pm[:])
                nc.sync.dma_start(
                    y_d[r0:r0 + 128, q * 2048:(q + 1) * 2048], ob[:])

            return {"pa": pa, "stage": stage, "out": out}

        t0 = make_tile(0)
        t1 = make_tile(1)
        # strict round-robin over (tile, half): engine queues are in-order
        # FIFOs, so each batch's inputs were produced >=3 batches earlier.
        t0["pa"](0)
        t0["pa"](1)
        t1["pa"](0)
        t1["pa"](1)
        for s in (9, 10):
            t0["stage"](s, 0)
            t0["stage"](s, 1)
            t1["stage"](s, 0)
            t1["stage"](s, 1)
        # s12 matmuls (both halves) read blocks 16..31 = s11's h1 output,
        # so emit the h1 batches first to maximize slack.
        t0["stage"](11, 1)
        t1["stage"](11, 1)
        t0["stage"](11, 0)
        t1["stage"](11, 0)
        t0["stage"](12, 0)
        t1["stage"](12, 0)
        t0["stage"](12, 1)
        t1["stage"](12, 1)
        # out quarters: q0/q2 need s12.h0 (lo/hi), q1/q3 need s12.h1
        for q in (0, 2, 1, 3):
            t0["out"](q)
            t1["out"](q)

    nc.compile()
    return nc


def kernel(x: np.ndarray, weights: np.ndarray) -> np.ndarray:
    x = np.asarray(x, dtype=np.float32)
    w = np.asarray(weights, dtype=np.float32)
    xb = np.ascontiguousarray(x.astype(NPBF))
    if "nc" not in _CACHE:
        _CACHE["nc"] = _build_program()
    nc = _CACHE["nc"]
    cwa, cwd = _host_consts(w)
    in_maps = [
        {"x": xb[ci * B_CORE:(ci + 1) * B_CORE], "cwa": cwa, "cwd": cwd}
        for ci in range(NCORES)
    ]
    res = run_bass_kernel_spmd(nc, in_maps, list(range(NCORES)))
    _CACHE["last_results"] = res
    t = np.concatenate([res.results[ci]["y"] for ci in range(NCORES)], axis=0)
    # y row layout: 32 x [re_block(128) | im_block(128)], block-major
    t = t.astype(np.float32).reshape(B, NBLK, 2, 128)
    t = (t[:, :, 0, :] + 1j * t[:, :, 1, :]).astype(np.complex64)
    t = t.reshape(B, IN_F)
    return np.concatenate([t, -t], axis=1)           # [2048, 8192]
